# revision 20
# baseline (speedup 1.0000x reference)
"""Lowpass (leaky integrator) scan kernel for Trainium2, 8 NeuronCores.

Recurrence (per feature n, per batch b):
    a_n = exp(-dt / max(tau_n, 1e-8))
    x_t = a_n * x_{t-1} + (1 - a_n) * u_t,   x_{-1} = initial_level_n

Strategy:
  - Data-parallel over batch: 32 batches -> 4 per core, no collectives.
  - Shard layout: each core's slice is staged feature-major [BC, N, T]
    (transposed at the host shard/unshard boundary), so features (N=128)
    sit on SBUF partitions and time runs along the free dimension. Every
    DMA is then fully contiguous (4-16KB runs per partition) and the
    native VectorEngine tensor_tensor_scan instruction evaluates the
    recurrence (state = a*state + u) along time directly at its full
    rate; the scan stream is the critical path.
  - We scan the rescaled variable z_t = a*z_{t-1} + u_t with
    z_{-1} = x0/(1-a); the ScalarEngine applies x = (1-a)*z with a
    per-partition scale on the way out.
  - Loads go out on SyncE's HWDGE queue, stores on ScalarE's, so the two
    streams don't FIFO behind each other.
  - The [1,128] filter coefficients (a, 1-a, x0/(1-a)) are precomputed
    host-side during sharding.
"""

import numpy as np
from contextlib import ExitStack

import concourse.bacc as bacc
import concourse.mybir as mybir
import concourse.tile as tile
from concourse.bass_utils import run_bass_kernel_spmd

DT = 0.001
B, T, N = 32, 4096, 128
NCORES = 8
BC = B // NCORES      # batches per core
TB = 512              # time columns per DMA/scale block
SC = 512              # time columns per scan instruction
NH = TB // SC         # scans per block
NBLK = T // TB        # blocks per batch

_F32 = mybir.dt.float32


def build_nc():
    nc = bacc.Bacc("TRN2", target_bir_lowering=False, debug=False)
    u = nc.declare_dram_parameter("u", [BC, N, T], _F32, isOutput=False)
    a_b_in = nc.declare_dram_parameter("a_b", [N, SC], _F32, isOutput=False)
    oma_in = nc.declare_dram_parameter("oma", [1, N], _F32, isOutput=False)
    z0_in = nc.declare_dram_parameter("z0", [1, N], _F32, isOutput=False)
    y = nc.declare_dram_parameter("y", [BC, N, T], _F32, isOutput=True)

    with tile.TileContext(nc) as tc, ExitStack() as ctx:
        const = ctx.enter_context(tc.tile_pool(name="const", bufs=1))
        in_pool = ctx.enter_context(tc.tile_pool(name="uin", bufs=8))
        z_pool = ctx.enter_context(tc.tile_pool(name="z", bufs=9))
        zs_pool = ctx.enter_context(tc.tile_pool(name="zs", bufs=6))

        # Constants ride ScalarE's HWDGE queue, which is empty at start,
        # so they load in parallel with the first input block on SyncE's.
        oma_col = const.tile([128, 1], _F32)
        z0_col = const.tile([128, 1], _F32)
        a_bcast = const.tile([128, SC], _F32)
        nc.scalar.dma_start(oma_col[:], oma_in[:].rearrange("o n -> n o"))
        nc.scalar.dma_start(z0_col[:], z0_in[:].rearrange("o n -> n o"))
        nc.scalar.dma_start(a_bcast[:], a_b_in[:])

        prev = [None] * BC
        for kb in range(NBLK):
            for b in range(BC):
                ut = in_pool.tile([128, TB], _F32, name="ut")
                nc.sync.dma_start(ut[:], u[b, :, kb * TB:(kb + 1) * TB])

                z = z_pool.tile([128, TB], _F32, name="z")
                for h in range(NH):
                    if h == 0:
                        init = z0_col[:, 0:1] if kb == 0 else prev[b][:, TB - 1:TB]
                    else:
                        init = z[:, h * SC - 1:h * SC]
                    nc.vector.tensor_tensor_scan(
                        z[:, h * SC:(h + 1) * SC], a_bcast[:],
                        ut[:, h * SC:(h + 1) * SC], init,
                        mybir.AluOpType.mult, mybir.AluOpType.add,
                    )
                prev[b] = z

                # x = (1-a) * z on the ScalarEngine (per-partition scale);
                # stores ride ScalarE's HWDGE queue, separate from loads.
                zs = zs_pool.tile([128, TB], _F32, name="zs")
                nc.scalar.mul(zs[:], z[:], oma_col[:, 0:1])
                nc.scalar.dma_start(y[b, :, kb * TB:(kb + 1) * TB], zs[:])
    nc.compile()
    return nc


_NC = None


def _get_nc():
    global _NC
    if _NC is None:
        _NC = build_nc()
    return _NC


def make_in_maps(inputs, initial_level, tau):
    # Shard layout: feature-major [BC, N, T] per core (contiguous DMA on
    # device); the transpose happens here at the shard boundary. The tiny
    # [1,128] filter coefficients are precomputed on the host.
    inputs_t = np.ascontiguousarray(
        np.asarray(inputs, dtype=np.float32).transpose(0, 2, 1)
    )
    tau = np.asarray(tau, dtype=np.float32)
    x0 = np.asarray(initial_level, dtype=np.float32)
    # fp32 exp via jax-on-CPU so `a` is bit-identical to the reference's;
    # a 1-ulp difference here is amplified by a^t over long horizons.
    try:
        import jax

        with jax.default_device(jax.local_devices(backend="cpu")[0]):
            a = np.asarray(
                jax.numpy.exp(-DT / jax.numpy.maximum(tau, 1e-8)),
                dtype=np.float32,
            )
    except Exception:
        a = np.exp(-np.float32(DT) / np.maximum(tau, np.float32(1e-8))).astype(
            np.float32
        )
    oma = (np.float32(1.0) - a).astype(np.float32)
    z0 = (x0 / oma).astype(np.float32)
    a_b = np.ascontiguousarray(np.broadcast_to(a.reshape(N, 1), (N, SC)))
    return [
        {
            "u": inputs_t[i * BC:(i + 1) * BC],
            "a_b": a_b,
            "oma": oma,
            "z0": z0,
        }
        for i in range(NCORES)
    ]


def kernel(inputs, initial_level, tau):
    nc = _get_nc()
    in_maps = make_in_maps(inputs, initial_level, tau)
    res = run_bass_kernel_spmd(nc, in_maps, list(range(NCORES))).results
    out_t = np.concatenate([res[i]["y"] for i in range(NCORES)], axis=0)
    return np.ascontiguousarray(out_t.transpose(0, 2, 1))


# revision 21
# speedup vs baseline: 1.0702x; 1.0702x over previous
"""Lowpass (leaky integrator) scan kernel for Trainium2, 8 NeuronCores.

Recurrence (per feature n, per batch b):
    a_n = exp(-dt / max(tau_n, 1e-8))
    x_t = a_n * x_{t-1} + (1 - a_n) * u_t,   x_{-1} = initial_level_n

Strategy:
  - Data-parallel over batch: 32 batches -> 4 per core, no collectives.
  - Shard layout: each core's slice is staged feature-major [BC, N, T]
    (transposed at the host shard/unshard boundary), so features (N=128)
    sit on SBUF partitions and time runs along the free dimension. Every
    DMA is then fully contiguous (4-16KB runs per partition) and the
    native VectorEngine tensor_tensor_scan instruction evaluates the
    recurrence (state = a*state + u) along time directly at its full
    rate; the scan stream is the critical path.
  - We scan the rescaled variable z_t = a*z_{t-1} + u_t with
    z_{-1} = x0/(1-a); the ScalarEngine applies x = (1-a)*z with a
    per-partition scale on the way out.
  - Loads go out on SyncE's HWDGE queue, stores on ScalarE's, so the two
    streams don't FIFO behind each other.
  - The [1,128] filter coefficients (a, 1-a, x0/(1-a)) are precomputed
    host-side during sharding.
"""

import numpy as np
from contextlib import ExitStack

import concourse.bacc as bacc
import concourse.mybir as mybir
import concourse.tile as tile
from concourse.bass_utils import run_bass_kernel_spmd

DT = 0.001
B, T, N = 32, 4096, 128
NCORES = 8
BC = B // NCORES      # batches per core
TB = 1024             # time columns per DMA/scale block
SC = 512              # time columns per scan instruction
NH = TB // SC         # scans per block
NBLK = T // TB        # blocks per batch

_F32 = mybir.dt.float32


def build_nc():
    nc = bacc.Bacc("TRN2", target_bir_lowering=False, debug=False)
    u = nc.declare_dram_parameter("u", [BC, N, T], _F32, isOutput=False)
    a_b_in = nc.declare_dram_parameter("a_b", [N, SC], _F32, isOutput=False)
    oma_in = nc.declare_dram_parameter("oma", [1, N], _F32, isOutput=False)
    z0_in = nc.declare_dram_parameter("z0", [1, N], _F32, isOutput=False)
    y = nc.declare_dram_parameter("y", [BC, N, T], _F32, isOutput=True)

    with tile.TileContext(nc) as tc, ExitStack() as ctx:
        const = ctx.enter_context(tc.tile_pool(name="const", bufs=1))
        in_pool = ctx.enter_context(tc.tile_pool(name="uin", bufs=8))
        z_pool = ctx.enter_context(tc.tile_pool(name="z", bufs=9))
        zs_pool = ctx.enter_context(tc.tile_pool(name="zs", bufs=6))

        # First input block goes out on the queue ahead of everything else.
        ut0 = in_pool.tile([128, TB], _F32, name="ut")
        nc.sync.dma_start(ut0[:], u[0, :, 0:TB])

        oma_col = const.tile([128, 1], _F32)
        z0_col = const.tile([128, 1], _F32)
        a_bcast = const.tile([128, SC], _F32)
        # Constants ride ScalarE's HWDGE queue (empty at start) so they
        # load in parallel with the first input block on SyncE's queue.
        nc.scalar.dma_start(oma_col[:], oma_in[:].rearrange("o n -> n o"))
        nc.scalar.dma_start(z0_col[:], z0_in[:].rearrange("o n -> n o"))
        nc.scalar.dma_start(a_bcast[:], a_b_in[:])

        prev = [None] * BC
        for kb in range(NBLK):
            for b in range(BC):
                if kb == 0 and b == 0:
                    ut = ut0
                else:
                    ut = in_pool.tile([128, TB], _F32, name="ut")
                    nc.sync.dma_start(ut[:], u[b, :, kb * TB:(kb + 1) * TB])

                z = z_pool.tile([128, TB], _F32, name="z")
                for h in range(NH):
                    if h == 0:
                        init = z0_col[:, 0:1] if kb == 0 else prev[b][:, TB - 1:TB]
                    else:
                        init = z[:, h * SC - 1:h * SC]
                    nc.vector.tensor_tensor_scan(
                        z[:, h * SC:(h + 1) * SC], a_bcast[:],
                        ut[:, h * SC:(h + 1) * SC], init,
                        mybir.AluOpType.mult, mybir.AluOpType.add,
                    )
                prev[b] = z

                # x = (1-a) * z on the ScalarEngine (per-partition scale);
                # stores ride ScalarE's HWDGE queue, separate from loads.
                last = kb == NBLK - 1 and b == BC - 1
                if not last:
                    zs = zs_pool.tile([128, TB], _F32, name="zs")
                    nc.scalar.mul(zs[:], z[:], oma_col[:, 0:1])
                    nc.scalar.dma_start(y[b, :, kb * TB:(kb + 1) * TB], zs[:])
                else:
                    # split the epilogue of the final block so the first
                    # half's scale/store overlaps the last scan
                    for h in range(NH):
                        zs = zs_pool.tile([128, SC], _F32, name="zsl")
                        nc.scalar.mul(
                            zs[:], z[:, h * SC:(h + 1) * SC], oma_col[:, 0:1]
                        )
                        nc.scalar.dma_start(
                            y[b, :, kb * TB + h * SC:kb * TB + (h + 1) * SC],
                            zs[:],
                        )
    nc.compile()
    return nc


_NC = None


def _get_nc():
    global _NC
    if _NC is None:
        _NC = build_nc()
    return _NC


def make_in_maps(inputs, initial_level, tau):
    # Shard layout: feature-major [BC, N, T] per core (contiguous DMA on
    # device); the transpose happens here at the shard boundary. The tiny
    # [1,128] filter coefficients are precomputed on the host.
    inputs_t = np.ascontiguousarray(
        np.asarray(inputs, dtype=np.float32).transpose(0, 2, 1)
    )
    tau = np.asarray(tau, dtype=np.float32)
    x0 = np.asarray(initial_level, dtype=np.float32)
    # fp32 exp via jax-on-CPU so `a` is bit-identical to the reference's;
    # a 1-ulp difference here is amplified by a^t over long horizons.
    try:
        import jax

        with jax.default_device(jax.local_devices(backend="cpu")[0]):
            a = np.asarray(
                jax.numpy.exp(-DT / jax.numpy.maximum(tau, 1e-8)),
                dtype=np.float32,
            )
    except Exception:
        a = np.exp(-np.float32(DT) / np.maximum(tau, np.float32(1e-8))).astype(
            np.float32
        )
    oma = (np.float32(1.0) - a).astype(np.float32)
    z0 = (x0 / oma).astype(np.float32)
    a_b = np.ascontiguousarray(np.broadcast_to(a.reshape(N, 1), (N, SC)))
    return [
        {
            "u": inputs_t[i * BC:(i + 1) * BC],
            "a_b": a_b,
            "oma": oma,
            "z0": z0,
        }
        for i in range(NCORES)
    ]


def kernel(inputs, initial_level, tau):
    nc = _get_nc()
    in_maps = make_in_maps(inputs, initial_level, tau)
    res = run_bass_kernel_spmd(nc, in_maps, list(range(NCORES))).results
    out_t = np.concatenate([res[i]["y"] for i in range(NCORES)], axis=0)
    return np.ascontiguousarray(out_t.transpose(0, 2, 1))
